# revision 1
# baseline (speedup 1.0000x reference)
"""Trainium2 Bass kernel for nn_Attention_2216203124924 (sparse/varlen GQA attention).

Full computation:
  xq/xk/xv = x @ {wq,wk,wv}.T ; per-head RMSNorm(q,k) ; RoPE via
  rope_cache[positions] ; GQA repeat ; per-segment causal attention
  (segments from cu_seqlens) ; out @ wo.T

Distribution (8 NeuronCores, tensor-parallel over heads):
  core c owns q-heads [4c,4c+4) and kv-head c (GQA groups align),
  wo is row-sharded; each core emits a partial [2048,4096] bf16 output and
  the host sums the 8 partials in f32.

On-device layout is "transposed" ([feature, seq]) throughout so the
contraction dim always sits on SBUF partitions. The kernel is a software
pipeline over 512-column m-chunks, structured to keep the PE dense (no
>3.4us idle windows, so the HAM clock gate stays at 8/8):

  Phase 1: per chunk, weight-stationary qkv projection (PSUM accumulators
  over 32 contraction tiles) runs dense on the PE while the PREVIOUS
  chunk's RMS stats + RoPE (DVE chains, host-gathered cos/sin, PE
  swap-half permutation) and V transposes execute on the other engines.

  Phase 2: per chunk, scoresT = kT.T @ q per key tile, unnormalized exp
  (scores are O(1)), compile-time segment mask plan, PV accumulated in
  PSUM; the softmax denominator is a gpsimd partition_all_reduce (no PSUM
  banks, no PE); normalization on DVE. The previous chunk's output
  projection is issued after each chunk's attention so its dense matmuls
  fill the attention chain gaps. Chunk 0's attention overlaps the last
  rope block via early release of the projection PSUM pool.

All matmul operands are bf16 or f32r (full PE rate). The segment/causal
structure from cu_seqlens and the rope gather by positions are resolved on
the host at build time; the NEFF is specialized to them.
"""

import os
import sys

import numpy as np

for _p in ("/opt/trn_rl_repo",):
    if os.path.isdir(_p) and _p not in sys.path:
        sys.path.insert(0, _p)

S = 2048
D = 4096
HD = 128
HALF = 64
N_HEADS = 32
N_KV = 8
NCORES = 8
QH = N_HEADS // NCORES          # 4 q heads per core
NO = QH + 2                     # o-tiles per core in qkv projection: q0..q3, k, v
DT = D // 128                   # 32 contraction tiles
MC = S // 512                   # 4 m-chunks of 512
NT = S // 128                   # 16 key tiles
EPS = 1e-6
SCALE = HD ** -0.5

LAST_RESULT = None  # BassKernelResults of the most recent run (for test harness)


def _attention_plan(cu_seqlens):
    """Compile-time mask plan from cu_seqlens.

    Returns (plan, mask_pack):
      plan[mc] = list of (nt, w0, w1, mask_ops); w0/w1 are column offsets
      (multiples of 128, relative to the 512-wide m-chunk) of the contiguous
      valid window; mask_ops = [(j, kind, idx)] for 128-col subtiles needing
      a multiplicative 0/1 mask: kind 'tri' uses the shared causal triangle,
      kind 'host' uses mask_pack[:, idx*128:(idx+1)*128].
    """
    idx = np.arange(S)
    seg = np.searchsorted(np.asarray(cu_seqlens), idx, side="right") - 1
    mask_qk = (seg[:, None] == seg[None, :]) & (idx[:, None] >= idx[None, :])
    mask_t = mask_qk.T  # [n, m]

    plan = []
    tiles = []
    tile_ids = {}
    for mc in range(MC):
        entries = []
        for nt in range(NT):
            blk = mask_t[nt * 128:(nt + 1) * 128, mc * 512:(mc + 1) * 512]
            if not blk.any():
                continue
            js = [j for j in range(4) if blk[:, j * 128:(j + 1) * 128].any()]
            jlo, jhi = min(js), max(js)
            assert js == list(range(jlo, jhi + 1)), "valid window not contiguous"
            mops = []
            for j in range(jlo, jhi + 1):
                sub = blk[:, j * 128:(j + 1) * 128]
                if sub.all():
                    continue
                m0g = mc * 512 + j * 128
                n0g = nt * 128
                if m0g == n0g and np.array_equal(
                    sub, idx[:128][None, :] >= idx[:128][:, None]
                ):
                    mops.append((j, "tri", -1))
                else:
                    key = sub.tobytes()
                    if key not in tile_ids:
                        tile_ids[key] = len(tiles)
                        tiles.append(sub.astype(np.float32))
                    mops.append((j, "host", tile_ids[key]))
            entries.append((nt, jlo * 128, (jhi + 1) * 128, mops))
        assert entries, "every query row attends to at least itself"
        plan.append(entries)

    if tiles:
        mask_pack = np.concatenate(tiles, axis=1)
    else:
        mask_pack = np.zeros((128, 128), dtype=np.float32)
    return plan, np.ascontiguousarray(mask_pack)


def _build_graph(plan, n_mask_cols):
    import concourse.bass as bass  # noqa: PLC0415
    import concourse.mybir as mybir  # noqa: PLC0415
    import concourse.tile as tile  # noqa: PLC0415
    from concourse import bacc, bass_isa  # noqa: PLC0415
    from contextlib import ExitStack  # noqa: PLC0415

    f32 = mybir.dt.float32
    f32r = mybir.dt.float32r
    bf16 = mybir.dt.bfloat16
    AF = mybir.ActivationFunctionType

    nc = bacc.Bacc()
    # all inputs are host-pretiled so every DMA reads one contiguous block
    xT_p = nc.declare_dram_parameter("xT", [DT * MC * 128, 512], bf16, isOutput=False)
    wqkv_p = nc.declare_dram_parameter("w_qkv", [DT * 128, NO * 128], bf16, isOutput=False)
    wo_p = nc.declare_dram_parameter("w_o", [4 * 128, QH * D // 4], bf16, isOutput=False)
    cs_p = nc.declare_dram_parameter("cs", [4 * MC * 128, 512], bf16, isOutput=False)
    consts_p = nc.declare_dram_parameter("consts", [128, 2 * 128], f32r, isOutput=False)
    constsb_p = nc.declare_dram_parameter(
        "consts_bf", [128, 3 * 128 + n_mask_cols], bf16, isOutput=False
    )
    out_p = nc.declare_dram_parameter("out", [S, D], bf16, isOutput=True)

    with tile.TileContext(nc) as tc, ExitStack() as ctx:
        const = ctx.enter_context(tc.tile_pool(name="const", bufs=1))
        persist = ctx.enter_context(tc.tile_pool(name="persist", bufs=1))
        # rope/stats small-matmul banks; outlives phase 1 — phase 2's output
        # projection reuses these two banks for its PSUM accumulators
        psm = ctx.enter_context(tc.tile_pool(name="smpsum", bufs=2, space="PSUM"))

        consts = const.tile([128, 2 * 128], f32r)
        ones_col = consts[:, 0:1]
        ones_row = consts[0:1, 0:128]
        sca_row = consts[0:1, 128:256]  # all = HD**0.5 (divide-by folds attn scale)

        constsb = const.tile([128, 3 * 128 + n_mask_cols], bf16)
        ones_col_bf = constsb[:, 0:1]
        swp_bf = constsb[:, 128:256]    # swap-halves permutation
        ident_bf = constsb[:, 256:384]  # identity (for PE transpose)
        mask_bf = constsb[:, 384:]

        eps_col = const.tile([128, 1], f32)
        nc.gpsimd.memset(eps_col[:], EPS)

        # full-S persistents: rope'd Q heads and K, transposed V (all bf16)
        kT = persist.tile([128, S], bf16)
        v_sb = persist.tile([128, S], bf16)
        wo_sb = persist.tile([128, QH * D], bf16)
        qbf = [persist.tile([128, S], bf16, tag=f"qbf{h}", name=f"qbf{h}") for h in range(QH)]

        # rotating per-(head, m-chunk) attention-output tiles
        pat = ctx.enter_context(tc.tile_pool(name="attn", bufs=3))

        # consts ride the slow gpsimd queue — nothing needs them before
        # ~50us, and keeping scalar clear lets weight chunk 0 land first
        nc.gpsimd.dma_start(consts[:], consts_p[:])
        nc.gpsimd.dma_start(constsb[:], constsb_p[:])

        with ExitStack() as s1:
            # small phase-1 pools first (they outlive the weight/x pools:
            # the last rope block runs during phase 2)
            pcs = s1.enter_context(tc.tile_pool(name="csstream", bufs=2))
            pqk = s1.enter_context(tc.tile_pool(name="qkvrot", bufs=2))
            prs = s1.enter_context(tc.tile_pool(name="rsq", bufs=3))
            pscr = s1.enter_context(tc.tile_pool(name="scratch", bufs=2))

            s1w = s1.enter_context(ExitStack())
            pw = s1w.enter_context(tc.tile_pool(name="wqkv", bufs=1))
            w_sb = pw.tile([128, NO * DT * 128], bf16)

            def w_fetch(d):
                eng = nc.gpsimd if d % 4 == 3 else nc.scalar
                eng.dma_start(
                    w_sb[:, d * NO * 128:(d + 1) * NO * 128],
                    wqkv_p[d * 128:(d + 1) * 128, :],
                )

            # d=0 split per-o so the very first matmul starts ~3us earlier
            for o in range(NO):
                nc.scalar.dma_start(
                    w_sb[:, o * 128:(o + 1) * 128],
                    wqkv_p[0:128, o * 128:(o + 1) * 128],
                )
            for d in range(1, DT):
                w_fetch(d)

            px = s1w.enter_context(tc.tile_pool(name="xstream", bufs=8))

            qkv_hist = {}
            cst_hist = {}
            rsq_hist = {}

            def stats_block(mc):
                """RMS stats (sum-of-squares matmul + scalar Sqrt) for chunk
                mc. All Sqrt activations are issued in phase 1 so the scalar
                activation table never thrashes against phase 2's Exp."""
                qkv = qkv_hist[mc]
                rsqs = []
                for o in range(QH + 1):
                    sq = pscr.tile([128, 512], f32r, tag="sq", name="sq")
                    nc.vector.tensor_mul(sq[:], qkv[o][:], qkv[o][:])
                    ss = psm.tile([1, 512], f32, tag="sm", name="ss", padded_shape=[128, 512])
                    nc.tensor.matmul(ss[:], ones_col, sq[:], start=True, stop=True)
                    rsq = prs.tile([1, 512], f32r, tag="rsq", name="rsq", bufs=6)
                    nc.scalar.activation(
                        rsq[:], ss[:], AF.Sqrt, bias=eps_col[0:1, :], scale=1.0 / HD,
                    )
                    rsqs.append(rsq)
                rsq_hist[mc] = rsqs

            def rot_block(mc):
                """RoPE rotation + V transposes for chunk mc (no scalar ops)."""
                msl = slice(mc * 512, (mc + 1) * 512)
                qkv = qkv_hist.pop(mc)
                cst = cst_hist.pop(mc)
                rsqs = rsq_hist.pop(mc)
                for o in range(QH + 1):
                    csb = 0 if o < QH else 2
                    row = sca_row if o < QH else ones_row
                    bp = psm.tile([128, 512], f32, tag="sm", name="bp")
                    nc.tensor.matmul(bp[:], swp_bf, qkv[o][:], start=True, stop=True)
                    t1 = pscr.tile([128, 512], f32, tag="t1", name="t1")
                    nc.vector.tensor_mul(t1[:], qkv[o][:], cst[csb][:])
                    t2 = pscr.tile([128, 512], f32, tag="t2", name="t2")
                    nc.vector.tensor_mul(t2[:], bp[:], cst[csb + 1][:])
                    nc.vector.tensor_add(t1[:], t1[:], t2[:])
                    bc = psm.tile([128, 512], f32, tag="sm", name="bc")
                    nc.tensor.matmul(bc[:], row, rsqs[o][:], start=True, stop=True)
                    rrb = pscr.tile([128, 512], f32, tag="rrb", name="rrb")
                    nc.vector.reciprocal_approx_fast(out=rrb[:], in_=bc[:])
                    dst = qbf[o][:, msl] if o < QH else kT[:, msl]
                    nc.vector.tensor_mul(dst, t1[:], rrb[:])

                for k in range(4):
                    nt = mc * 4 + k
                    tp = psm.tile([128, 128], bf16, tag="sm", name="tp")
                    nc.tensor.transpose(tp[:], qkv[QH + 1][:, k * 128:(k + 1) * 128], ident_bf)
                    nc.vector.tensor_copy(v_sb[:, nt * 128:(nt + 1) * 128], tp[:])

            # ---- phase 1: projection d-loops, rope pipelined one chunk behind
            with ExitStack() as s1p:
                pq = s1p.enter_context(tc.tile_pool(name="qkvpsum", bufs=1, space="PSUM"))
                for mc in range(MC):
                    msl = slice(mc * 512, (mc + 1) * 512)
                    accs = [
                        pq.tile([128, 512], f32, tag=f"acc{o}", name=f"acc{o}")
                        for o in range(NO)
                    ]
                    for d in range(DT):
                        xt = px.tile([128, 512], bf16, tag="xt")
                        xeng = nc.sync if (mc == 0 or d % 2 == 0) else nc.gpsimd
                        xr0 = (d * MC + mc) * 128
                        xeng.dma_start(xt[:], xT_p[xr0:xr0 + 128, :])
                        for o in range(NO):
                            woff = (d * NO + o) * 128
                            nc.tensor.matmul(
                                accs[o][:],
                                w_sb[:, woff:woff + 128],
                                xt[:],
                                start=(d == 0),
                                stop=(d == DT - 1),
                            )

                    # PSUM -> bf16 casts split across DVE and scalar (Copy
                    # loads no ACT table) so the acc banks free ~2x faster
                    # at the chunk seam
                    qkv = []
                    for o in range(NO):
                        t = pqk.tile([128, 512], bf16, tag=f"qk{o}", name=f"qk{o}")
                        if o % 2 == 0:
                            nc.vector.tensor_copy(t[:], accs[o][:])
                        else:
                            nc.scalar.activation(t[:], accs[o][:], AF.Copy)
                        qkv.append(t)
                    qkv_hist[mc] = qkv

                    # cs for this chunk (consumed by rot_block under the next
                    # chunk's d-loop; issued after the casts so they never
                    # queue behind DMA issues on scalar)
                    cst = {}
                    for ci in range(4):
                        t = pcs.tile([128, 512], bf16, tag=f"cs{ci}", name=f"cs{ci}")
                        r0 = (ci * MC + mc) * 128
                        nc.scalar.dma_start(t[:], cs_p[r0:r0 + 128, :])
                        cst[ci] = t
                    cst_hist[mc] = cst

                    if mc >= 1:
                        stats_block(mc - 1)
                        rot_block(mc - 1)
                    if mc == 2:
                        # wo prefetch after the startup DMA crunch is over
                        woch = QH * D // 4
                        for wci in range(4):
                            nc.scalar.dma_start(
                                wo_sb[:, wci * woch:(wci + 1) * woch],
                                wo_p[wci * 128:(wci + 1) * 128, :],
                            )
                # last chunk's stats now: every scalar Sqrt precedes phase
                # 2's first Exp, so the activation table loads only twice.
                stats_block(MC - 1)
            # pq released here: phase-2 PSUM pools take its banks, so chunk
            # 0's attention can overlap the final rope block below. The
            # weight/x SBUF pools close too, making room for phase 2.
            s1w.close()

            # ---------------- phase 2: attention + output projection ----------------
            with ExitStack() as s2:
                psco = s2.enter_context(tc.tile_pool(name="scpsum", bufs=2, space="PSUM"))
                pov = s2.enter_context(tc.tile_pool(name="ovpsum", bufs=2, space="PSUM"))
                # den + bc2 share a 2-slot family so consecutive heads'
                # denominator groups overlap instead of serializing
                pden = s2.enter_context(tc.tile_pool(name="denpsum", bufs=2, space="PSUM"))
                pex = s2.enter_context(tc.tile_pool(name="exsbuf", bufs=3))
                pnr = s2.enter_context(tc.tile_pool(name="nrsbuf", bufs=2))
                pys = s2.enter_context(tc.tile_pool(name="ysbuf", bufs=3))

                def attn_block(mc, den_pool=None, sprinkle=False):
                    den_pool = den_pool or pden
                    bc_pool = den_pool
                    entries = plan[mc]
                    n_ent = len(entries)
                    attnT = []
                    for h in range(QH):
                        ov = pov.tile([128, 512], f32, tag="ov")
                        den = den_pool.tile(
                            [1, 512], f32,
                            tag="dn" if den_pool is pden else "sm",
                            name="den", padded_shape=[128, 512],
                        )
                        for i, (nt, w0, w1, mops) in enumerate(entries):
                            nsl = slice(nt * 128, (nt + 1) * 128)
                            sc = psco.tile([128, 512], f32, tag="sc")
                            nc.tensor.matmul(
                                sc[:, w0:w1], kT[:, nsl],
                                qbf[h][:, mc * 512 + w0: mc * 512 + w1],
                                start=True, stop=True,
                            )
                            ex = pex.tile([128, 512], bf16, tag="ex")
                            nc.scalar.activation(ex[:, w0:w1], sc[:, w0:w1], AF.Exp)
                            for (j, kind, tix) in mops:
                                jsl = slice(j * 128, (j + 1) * 128)
                                if kind == "tri":
                                    # zero strictly-below-diagonal (m < n) entries
                                    nc.gpsimd.affine_select(
                                        out=ex[:, jsl], in_=ex[:, jsl],
                                        compare_op=mybir.AluOpType.is_ge,
                                        fill=0.0, base=0,
                                        pattern=[[1, 128]], channel_multiplier=-1,
                                    )
                                else:
                                    nc.vector.tensor_mul(
                                        ex[:, jsl], ex[:, jsl],
                                        mask_bf[:, tix * 128:(tix + 1) * 128],
                                    )
                            first = i == 0
                            last = i == n_ent - 1
                            nc.tensor.matmul(
                                ov[:, w0:w1], v_sb[:, nsl], ex[:, w0:w1],
                                start=first, stop=last, skip_group_check=True,
                            )
                            nc.tensor.matmul(
                                den[0:1, w0:w1], ones_col_bf, ex[:, w0:w1],
                                start=first, stop=last, skip_group_check=True,
                            )
                        den_sb = pnr.tile([1, 512], f32r, tag="den_sb")
                        nc.vector.tensor_copy(den_sb[:], den[:])
                        bc2 = bc_pool.tile(
                            [128, 512], f32,
                            tag="dn" if bc_pool is pden else "sm", name="bc2",
                        )
                        nc.tensor.matmul(bc2[:], ones_row, den_sb[:], start=True, stop=True)
                        rrb2 = pnr.tile([128, 512], f32, tag="rrb2")
                        nc.vector.reciprocal_approx_fast(out=rrb2[:], in_=bc2[:])
                        at = pat.tile([128, 512], bf16, tag=f"attnT{h}", name=f"attnT{h}")
                        nc.vector.tensor_mul(at[:], ov[:], rrb2[:])
                        attnT.append(at)
                    return attnT

                def outproj(mc, attnT):
                    for j in range(4):
                        mt = mc * 4 + j
                        tsl = slice(mt * 128, (mt + 1) * 128)
                        jsl = slice(j * 128, (j + 1) * 128)
                        ys = pys.tile([128, D], bf16, tag="ys", name="ys")
                        for ec in range(D // 512):
                            # yp reuses the rope small-matmul banks (rope is
                            # done before any output projection starts)
                            yp = psm.tile([128, 512], f32, tag="sm", name="yp")
                            for t in range(QH):
                                nc.tensor.matmul(
                                    yp[:],
                                    attnT[t][:, jsl],
                                    wo_sb[:, t * D + ec * 512: t * D + (ec + 1) * 512],
                                    start=(t == 0),
                                    stop=(t == QH - 1),
                                )
                            esl = slice(ec * 512, (ec + 1) * 512)
                            # Copy loads no ACT table, so scalar is safe to
                            # share with the in-flight Exp activations
                            if ec % 2 == 0:
                                nc.scalar.activation(ys[:, esl], yp[:], AF.Copy)
                            else:
                                nc.vector.tensor_copy(ys[:, esl], yp[:])
                        if mc == MC - 1:
                            half = D // 2
                            nc.sync.dma_start(out_p[tsl, 0:half], ys[:, 0:half])
                            nc.scalar.dma_start(out_p[tsl, half:D], ys[:, half:D])
                        else:
                            nc.sync.dma_start(out_p[tsl, :], ys[:])

                def ham_burst(n):
                    # >=3.4us of dependency-free back-to-back matmuls flips
                    # the HAM clock gate back to 8/8; the sparse attention
                    # transition otherwise runs 30-60us at half clock
                    for _ in range(n):
                        dummy = psm.tile([128, 512], f32, tag="sm", name="dummy")
                        nc.tensor.matmul(dummy[:], swp_bf, qbf[0][:, 0:512],
                                         start=True, stop=True)

                # last rope first: its Sqrt activations and small matmuls
                # drain before the first Exp, avoiding ACT-table thrash
                ham_burst(20)
                rot_block(MC - 1)
                ham_burst(16)
                attnT_hist = {0: attn_block(0)}
                ham_burst(16)
                attnT_hist[1] = attn_block(1)
                ham_burst(16)
                outproj(0, attnT_hist.pop(0))
                for mc in range(2, MC):
                    attnT_hist[mc] = attn_block(mc)
                    outproj(mc - 1, attnT_hist.pop(mc - 1))
                outproj(MC - 1, attnT_hist.pop(MC - 1))

    nc.finalize()
    return nc


def kernel(x, wq, wk, wv, wo, q_norm_w, k_norm_w, rope_cache, positions, cu_seqlens):
    global LAST_RESULT
    from concourse.bass_utils import run_bass_kernel_spmd  # noqa: PLC0415

    x = np.asarray(x, dtype=np.float32)
    wq = np.asarray(wq, dtype=np.float32)
    wk = np.asarray(wk, dtype=np.float32)
    wv = np.asarray(wv, dtype=np.float32)
    wo = np.asarray(wo, dtype=np.float32)
    q_norm_w = np.asarray(q_norm_w, dtype=np.float32)
    k_norm_w = np.asarray(k_norm_w, dtype=np.float32)
    rope_cache = np.asarray(rope_cache, dtype=np.float32)
    positions = np.asarray(positions)
    cu_seqlens = np.asarray(cu_seqlens)

    import ml_dtypes  # noqa: PLC0415

    # ---- host prep (shared) ----
    # x pretiled: tile (d, mc) = rows [(d*MC+mc)*128, +128) as one
    # contiguous [128, 512] block
    xT = np.ascontiguousarray(
        x[0].T.astype(ml_dtypes.bfloat16)
        .reshape(DT, 128, MC, 512).transpose(0, 2, 1, 3)
        .reshape(DT * MC * 128, 512)
    )

    pos = positions.reshape(-1)
    cs = rope_cache[pos]               # [S, HALF, 2]
    cosT = cs[:, :, 0].T               # [HALF, S]
    sinT = cs[:, :, 1].T
    cs1 = np.concatenate([cosT, cosT], axis=0)    # [128, S]
    cs2 = np.concatenate([-sinT, sinT], axis=0)

    def fold(w):
        w = w.reshape(HD, 1)
        wsw = np.concatenate([w[HALF:], w[:HALF]], axis=0)
        return cs1 * w, cs2 * wsw

    cs1q, cs2q = fold(q_norm_w)
    cs1k, cs2k = fold(k_norm_w)
    # pretiled: tile (ci, mc) = rows [(ci*MC+mc)*128, +128)
    cs_host = np.ascontiguousarray(
        np.concatenate([cs1q, cs2q, cs1k, cs2k], axis=1).astype(ml_dtypes.bfloat16)
        .reshape(128, 4, MC, 512).transpose(1, 2, 0, 3)
        .reshape(4 * MC * 128, 512)
    )

    plan, mask_pack = _attention_plan(cu_seqlens)

    consts_bf = np.zeros((128, 3 * 128 + mask_pack.shape[1]), dtype=np.float32)
    consts_bf[:, 0:128] = 1.0
    swp = np.zeros((128, 128), dtype=np.float32)
    swp[np.arange(128), (np.arange(128) + HALF) % 128] = 1.0
    consts_bf[:, 128:256] = swp
    consts_bf[:, 256:384] = np.eye(128, dtype=np.float32)
    consts_bf[:, 384:] = mask_pack
    consts_bf = consts_bf.astype(ml_dtypes.bfloat16)

    consts = np.zeros((128, 2 * 128), dtype=np.float32)
    consts[:, 0:128] = 1.0
    consts[:, 128:256] = 1.0 / SCALE

    # ---- per-core weight shards ----
    in_maps = []
    for c in range(NCORES):
        w_all = np.concatenate(
            [
                wq[c * QH * HD:(c + 1) * QH * HD],   # [512, D]
                wk[c * HD:(c + 1) * HD],             # [128, D]
                wv[c * HD:(c + 1) * HD],             # [128, D]
            ],
            axis=0,
        )  # [NO*128, D]
        # pretiled: chunk d = rows [d*128, +128) holding [128, NO*128]
        w_host = np.ascontiguousarray(
            w_all.reshape(NO, 128, DT, 128).transpose(3, 2, 0, 1)
            .reshape(128, DT, NO * 128).transpose(1, 0, 2)
            .reshape(DT * 128, NO * 128).astype(ml_dtypes.bfloat16)
        )
        wo_c = wo[:, c * QH * HD:(c + 1) * QH * HD].T  # [512, D]
        # pretiled: chunk wci = rows [wci*128, +128) holding [128, QH*D/4]
        wo_host = np.ascontiguousarray(
            wo_c.reshape(QH, 128, D).transpose(1, 0, 2)
            .reshape(128, 4, QH * D // 4).transpose(1, 0, 2)
            .reshape(4 * 128, QH * D // 4).astype(ml_dtypes.bfloat16)
        )
        in_maps.append(
            {
                "xT": xT,
                "w_qkv": w_host,
                "w_o": wo_host,
                "cs": cs_host,
                "consts": consts,
                "consts_bf": consts_bf,
            }
        )

    nc = _build_graph(plan, mask_pack.shape[1])
    res = run_bass_kernel_spmd(nc, in_maps, list(range(NCORES)))
    LAST_RESULT = res

    out = res.results[0]["out"].astype(np.float32)
    for c in range(1, NCORES):
        out = out + res.results[c]["out"].astype(np.float32)
    return out.reshape(1, S, D)



# revision 6
# speedup vs baseline: 1.0226x; 1.0226x over previous
"""Trainium2 Bass kernel for nn_Attention_2216203124924 (sparse/varlen GQA attention).

Full computation:
  xq/xk/xv = x @ {wq,wk,wv}.T ; per-head RMSNorm(q,k) ; RoPE via
  rope_cache[positions] ; GQA repeat ; per-segment causal attention
  (segments from cu_seqlens) ; out @ wo.T

Distribution (8 NeuronCores, tensor-parallel over heads):
  core c owns q-heads [4c,4c+4) and kv-head c (GQA groups align),
  wo is row-sharded; each core emits a partial [2048,4096] bf16 output and
  the host sums the 8 partials in f32.

On-device layout is "transposed" ([feature, seq]) so the contraction dim
always sits on SBUF partitions. The kernel is a single software pipeline
over 512-column chunks with NO phase barrier: for each chunk,
qkv projection (o-major: one PSUM accumulator at a time over 32
contraction tiles) -> RMS stats -> RoPE -> per-segment attention ->
output projection, all issued in one stream. The Tile scheduler's
priority order (= issue order) makes the next chunk's projection matmuls
the natural PE filler during the current chunk's attention dependency
stalls, so the PE stays dense and the HAM clock gate never re-throttles.

PE-offloads vs the obvious formulation:
  - RoPE swap-halves via two SBUF->SBUF DMAs (not a PE permute matmul)
  - rsqrt/softmax-denominator broadcasts via gpsimd partition_broadcast
    (not PE ones-outer-product matmuls)
  - attn scale and eps folded into the ACT Sqrt bias/scale
PSUM budget (8 banks): proj acc x2, scores x2, PV acc x1, outproj acc x2,
smalls (stats/transpose/den) x1.

All matmul operands are bf16 (full PE rate). The segment/causal structure
from cu_seqlens and the rope gather by positions are resolved on the host
at build time; the NEFF is specialized to them.
"""

import os
import sys

import numpy as np

for _p in ("/opt/trn_rl_repo",):
    if os.path.isdir(_p) and _p not in sys.path:
        sys.path.insert(0, _p)

S = 2048
D = 4096
HD = 128
HALF = 64
N_HEADS = 32
N_KV = 8
NCORES = 8
QH = N_HEADS // NCORES          # 4 q heads per core
NO = QH + 2                     # projection outputs per core: q0..q3, k, v
DT = D // 128                   # 32 contraction tiles
MC = S // 512                   # 4 m-chunks of 512
NT = S // 128                   # 16 key tiles
GD = 8                          # x d-tiles per DMA group
NG = DT // GD                   # 4 groups per chunk
WG = 4                          # w d-tiles per DMA group
EPS = 1e-6
SCALE = HD ** -0.5

LAST_RESULT = None  # BassKernelResults of the most recent run (for test harness)


def _attention_plan(cu_seqlens):
    """Compile-time mask plan from cu_seqlens.

    Returns (plan, mask_pack):
      plan[mc] = list of (nt, w0, w1, mask_ops); w0/w1 are column offsets
      (multiples of 128, relative to the 512-wide m-chunk) of the contiguous
      valid window; mask_ops = [(j, kind, idx)] for 128-col subtiles needing
      a multiplicative 0/1 mask: kind 'tri' uses a gpsimd affine_select,
      kind 'host' uses mask_pack[:, idx*128:(idx+1)*128].
    """
    idx = np.arange(S)
    seg = np.searchsorted(np.asarray(cu_seqlens), idx, side="right") - 1
    mask_qk = (seg[:, None] == seg[None, :]) & (idx[:, None] >= idx[None, :])
    mask_t = mask_qk.T  # [n, m]

    plan = []
    tiles = []
    tile_ids = {}
    for mc in range(MC):
        entries = []
        for nt in range(NT):
            blk = mask_t[nt * 128:(nt + 1) * 128, mc * 512:(mc + 1) * 512]
            if not blk.any():
                continue
            js = [j for j in range(4) if blk[:, j * 128:(j + 1) * 128].any()]
            jlo, jhi = min(js), max(js)
            assert js == list(range(jlo, jhi + 1)), "valid window not contiguous"
            mops = []
            for j in range(jlo, jhi + 1):
                sub = blk[:, j * 128:(j + 1) * 128]
                if sub.all():
                    continue
                m0g = mc * 512 + j * 128
                n0g = nt * 128
                if m0g == n0g and np.array_equal(
                    sub, idx[:128][None, :] >= idx[:128][:, None]
                ):
                    mops.append((j, "tri", -1))
                else:
                    key = sub.tobytes()
                    if key not in tile_ids:
                        tile_ids[key] = len(tiles)
                        tiles.append(sub.astype(np.float32))
                    mops.append((j, "host", tile_ids[key]))
            entries.append((nt, jlo * 128, (jhi + 1) * 128, mops))
        assert entries, "every query row attends to at least itself"
        plan.append(entries)

    if tiles:
        mask_pack = np.concatenate(tiles, axis=1)
    else:
        mask_pack = np.zeros((128, 128), dtype=np.float32)
    return plan, np.ascontiguousarray(mask_pack)


def _build_graph(plan, n_mask_cols):
    import concourse.bass as bass  # noqa: PLC0415
    import concourse.mybir as mybir  # noqa: PLC0415
    import concourse.tile as tile  # noqa: PLC0415
    from concourse import bacc, bass_isa  # noqa: PLC0415
    from contextlib import ExitStack  # noqa: PLC0415

    f32 = mybir.dt.float32
    bf16 = mybir.dt.bfloat16
    AF = mybir.ActivationFunctionType

    nc = bacc.Bacc()
    # all inputs host-pretiled so every DMA is one contiguous block per
    # partition (large descriptors, few instructions)
    xT_p = nc.declare_dram_parameter("xT", [MC * NG * 128, GD * 512], bf16, isOutput=False)
    wqkv_p = nc.declare_dram_parameter("w_qkv", [(DT // WG) * 128, WG * NO * 128], bf16, isOutput=False)
    wo_p = nc.declare_dram_parameter("w_o", [4 * 128, QH * D // 4], bf16, isOutput=False)
    cs_p = nc.declare_dram_parameter("cs", [MC * 128, 4 * 512], bf16, isOutput=False)
    constsb_p = nc.declare_dram_parameter(
        "consts_bf", [128, 2 * 128 + n_mask_cols], bf16, isOutput=False
    )
    out_p = nc.declare_dram_parameter("out", [S, D], bf16, isOutput=True)

    with tile.TileContext(nc) as tc, ExitStack() as ctx:
        const = ctx.enter_context(tc.tile_pool(name="const", bufs=1))
        persist = ctx.enter_context(tc.tile_pool(name="persist", bufs=1))

        constsb = const.tile([128, 2 * 128 + n_mask_cols], bf16)
        ones_col_bf = constsb[:, 0:1]
        ident_bf = constsb[:, 128:256]   # identity (for PE transpose)
        mask_bf = constsb[:, 256:]

        epsq = const.tile([1, 1], f32, name="epsq")
        epsk = const.tile([1, 1], f32, name="epsk")
        nc.gpsimd.memset(epsq[:], HD * EPS)
        nc.gpsimd.memset(epsk[:], EPS)

        # persistent weights + per-chunk K/V tiles
        w_sb = persist.tile([128, NO * DT * 128], bf16)
        wo_sb = persist.tile([128, QH * D], bf16)
        kt_c = [persist.tile([128, 512], bf16, name=f"kt{m}") for m in range(MC)]
        v_c = [persist.tile([128, 512], bf16, name=f"vt{m}") for m in range(MC)]

        # ---- startup DMAs ----
        # w groups alternate scalar/gpsimd queues so the first o-pass's tail
        # d-tiles arrive in ~9us instead of ~18us
        NWG = DT // WG
        for g in range(NWG):
            eng = nc.scalar if g % 2 == 0 else nc.gpsimd
            eng.dma_start(
                w_sb[:, g * WG * NO * 128:(g + 1) * WG * NO * 128],
                wqkv_p[g * 128:(g + 1) * 128, :],
            )
        nc.scalar.dma_start(constsb[:], constsb_p[:])
        woch = QH * D // 4
        for wci in range(4):
            nc.gpsimd.dma_start(
                wo_sb[:, wci * woch:(wci + 1) * woch],
                wo_p[wci * 128:(wci + 1) * 128, :],
            )

        # rotating pools
        pxt = ctx.enter_context(tc.tile_pool(name="xstream", bufs=5))
        pcs = ctx.enter_context(tc.tile_pool(name="csstream", bufs=2))
        pqk = ctx.enter_context(tc.tile_pool(name="qkvrot", bufs=2))
        psw = ctx.enter_context(tc.tile_pool(name="swap", bufs=2))
        psq = ctx.enter_context(tc.tile_pool(name="sq", bufs=1))
        prs = ctx.enter_context(tc.tile_pool(name="rs", bufs=2))
        prb = ctx.enter_context(tc.tile_pool(name="rb", bufs=1))
        pt = ctx.enter_context(tc.tile_pool(name="t12", bufs=1))
        pqb = ctx.enter_context(tc.tile_pool(name="qb", bufs=2))
        pex = ctx.enter_context(tc.tile_pool(name="ex", bufs=3))
        pdn = ctx.enter_context(tc.tile_pool(name="dn", bufs=2))
        pat = ctx.enter_context(tc.tile_pool(name="attn", bufs=2))
        pys = ctx.enter_context(tc.tile_pool(name="ys", bufs=2))

        # PSUM: 2+2+1+2+1 = 8 banks
        pacc = ctx.enter_context(tc.tile_pool(name="accpsum", bufs=2, space="PSUM"))
        psco = ctx.enter_context(tc.tile_pool(name="scpsum", bufs=2, space="PSUM"))
        pov = ctx.enter_context(tc.tile_pool(name="ovpsum", bufs=1, space="PSUM"))
        pyp = ctx.enter_context(tc.tile_pool(name="yppsum", bufs=2, space="PSUM"))
        psm = ctx.enter_context(tc.tile_pool(name="smpsum", bufs=1, space="PSUM"))

        def xt_fetch(mc, split_first=False):
            """DMA chunk mc's x tiles (NG groups of GD d-tiles) on sync."""
            grp = []
            for g in range(NG):
                t = pxt.tile([128, GD * 512], bf16, tag="xt", name=f"xt{mc}_{g}")
                r0 = (mc * NG + g) * 128
                if split_first and g == 0:
                    h = GD * 512 // 2
                    nc.sync.dma_start(t[:, 0:h], xT_p[r0:r0 + 128, 0:h])
                    nc.sync.dma_start(t[:, h:], xT_p[r0:r0 + 128, h:])
                else:
                    nc.sync.dma_start(t[:], xT_p[r0:r0 + 128, :])
                grp.append(t)
            return grp

        def cs_fetch(mc):
            t = pcs.tile([128, 4 * 512], bf16, tag="cs", name=f"cs{mc}")
            nc.scalar.dma_start(t[:], cs_p[mc * 128:(mc + 1) * 128, :])
            return t

        xt_cur = xt_fetch(0, split_first=True)
        cs_cur = cs_fetch(0)

        # o-pass order: q0, k, v first so attention's head-0 chain can start
        # while q1..q3 are still projecting
        O_ORDER = [0, QH, QH + 1, 1, 2, 3]

        for mc in range(MC):
            msl = slice(mc * 512, (mc + 1) * 512)
            if mc + 1 < MC:
                xt_nxt = xt_fetch(mc + 1)
                cs_nxt = cs_fetch(mc + 1)

            qbf = [None] * QH

            for oi, o in enumerate(O_ORDER):
                acc = pacc.tile([128, 512], f32, tag="acc", name=f"acc{mc}_{o}")
                for d in range(DT):
                    woff = (d * NO + o) * 128
                    nc.tensor.matmul(
                        acc[:],
                        w_sb[:, woff:woff + 128],
                        xt_cur[d // GD][:, (d % GD) * 512:(d % GD + 1) * 512],
                        start=(d == 0),
                        stop=(d == DT - 1),
                    )
                # PSUM -> bf16 cast, alternating DVE / ACT(Copy: no table)
                qkv = pqk.tile([128, 512], bf16, tag=f"qk{o}", name=f"qk{o}")
                if oi % 2 == 0:
                    nc.vector.tensor_copy(qkv[:], acc[:])
                else:
                    nc.scalar.activation(qkv[:], acc[:], AF.Copy)

                if o == QH + 1:
                    # V: transpose to [seq, hd] blocks
                    for k in range(4):
                        tp = psm.tile([128, 128], bf16, tag="sm", name="tp")
                        nc.tensor.transpose(
                            tp[:], qkv[:, k * 128:(k + 1) * 128], ident_bf
                        )
                        nc.vector.tensor_copy(v_c[mc][:, k * 128:(k + 1) * 128], tp[:])
                    continue

                # ---- RMS stats ----
                sq = psq.tile([128, 512], bf16, tag="sq", name="sq")
                nc.vector.tensor_mul(sq[:], qkv[:], qkv[:])
                ss = psm.tile([1, 512], f32, tag="sm", name="ss", padded_shape=[128, 512])
                nc.tensor.matmul(ss[:], ones_col_bf, sq[:], start=True, stop=True)
                rsq = prs.tile([1, 512], f32, tag="rsq", name="rsq")
                if o < QH:
                    # sqrt(ss + HD*eps) = sqrt(HD)*sqrt(ms+eps); recip folds
                    # the attention 1/sqrt(HD) scale into q's normalization
                    nc.scalar.activation(rsq[:], ss[:], AF.Sqrt, bias=epsq[:], scale=1.0)
                else:
                    nc.scalar.activation(rsq[:], ss[:], AF.Sqrt, bias=epsk[:], scale=1.0 / HD)
                nc.vector.reciprocal_approx_fast(out=rsq[:], in_=rsq[:])
                rrb = prb.tile([128, 512], f32, tag="rrb", name="rrb")
                nc.gpsimd.partition_broadcast(rrb[:], rsq[:], channels=128)

                # ---- RoPE ----
                # swap halves via SBUF->SBUF DMA (no PE permute)
                sw = psw.tile([128, 512], bf16, tag="sw", name="sw")
                nc.gpsimd.dma_start(sw[0:HALF, :], qkv[HALF:128, :])
                nc.gpsimd.dma_start(sw[HALF:128, :], qkv[0:HALF, :])
                csb = 0 if o < QH else 2
                t1 = pt.tile([128, 512], bf16, tag="t1", name="t1")
                nc.vector.tensor_mul(t1[:], qkv[:], cs_cur[:, csb * 512:(csb + 1) * 512])
                t2 = pt.tile([128, 512], bf16, tag="t2", name="t2")
                nc.vector.tensor_mul(t2[:], sw[:], cs_cur[:, (csb + 1) * 512:(csb + 2) * 512])
                nc.vector.tensor_add(t1[:], t1[:], t2[:])
                if o < QH:
                    qb = pqb.tile([128, 512], bf16, tag=f"qb{o}", name=f"qb{o}")
                    nc.vector.tensor_mul(qb[:], t1[:], rrb[:])
                    qbf[o] = qb
                else:
                    nc.vector.tensor_mul(kt_c[mc][:], t1[:], rrb[:])

            # ---------------- attention for chunk mc ----------------
            entries = plan[mc]
            n_ent = len(entries)
            attnT = []
            for h in range(QH):
                ov = pov.tile([128, 512], f32, tag="ov", name="ov")
                den = psm.tile([1, 512], f32, tag="sm", name="den", padded_shape=[128, 512])
                for i, (nt, w0, w1, mops) in enumerate(entries):
                    kc, ko = nt // 4, (nt % 4) * 128
                    sc = psco.tile([128, 512], f32, tag="sc", name="sc")
                    nc.tensor.matmul(
                        sc[:, w0:w1], kt_c[kc][:, ko:ko + 128],
                        qbf[h][:, w0:w1],
                        start=True, stop=True,
                    )
                    ex = pex.tile([128, 512], bf16, tag="ex", name="ex")
                    nc.scalar.activation(ex[:, w0:w1], sc[:, w0:w1], AF.Exp)
                    for (j, kind, tix) in mops:
                        jsl = slice(j * 128, (j + 1) * 128)
                        if kind == "tri":
                            # zero strictly-below-diagonal (m < n) entries
                            nc.gpsimd.affine_select(
                                out=ex[:, jsl], in_=ex[:, jsl],
                                compare_op=mybir.AluOpType.is_ge,
                                fill=0.0, base=0,
                                pattern=[[1, 128]], channel_multiplier=-1,
                            )
                        else:
                            nc.vector.tensor_mul(
                                ex[:, jsl], ex[:, jsl],
                                mask_bf[:, tix * 128:(tix + 1) * 128],
                            )
                    first = i == 0
                    last = i == n_ent - 1
                    nc.tensor.matmul(
                        ov[:, w0:w1], v_c[kc][:, ko:ko + 128], ex[:, w0:w1],
                        start=first, stop=last, skip_group_check=True,
                    )
                    nc.tensor.matmul(
                        den[0:1, w0:w1], ones_col_bf, ex[:, w0:w1],
                        start=first, stop=last, skip_group_check=True,
                    )
                den_sb = pdn.tile([1, 512], f32, tag="den_sb", name="den_sb")
                nc.vector.tensor_copy(den_sb[:], den[:])
                nc.vector.reciprocal_approx_fast(out=den_sb[:], in_=den_sb[:])
                rrb2 = prb.tile([128, 512], f32, tag="rrb2", name="rrb2")
                nc.gpsimd.partition_broadcast(rrb2[:], den_sb[:], channels=128)
                at = pat.tile([128, 512], bf16, tag=f"attnT{h}", name=f"attnT{h}")
                nc.vector.tensor_mul(at[:], ov[:], rrb2[:])
                attnT.append(at)

            # ---------------- output projection for chunk mc ----------------
            for j in range(4):
                mt = mc * 4 + j
                tsl = slice(mt * 128, (mt + 1) * 128)
                jsl = slice(j * 128, (j + 1) * 128)
                for half in range(2):
                    ys = pys.tile([128, D // 2], bf16, tag="ys", name="ys")
                    for eh in range(4):
                        ec = half * 4 + eh
                        yp = pyp.tile([128, 512], f32, tag="yp", name="yp")
                        for t in range(QH):
                            nc.tensor.matmul(
                                yp[:],
                                attnT[t][:, jsl],
                                wo_sb[:, t * D + ec * 512: t * D + (ec + 1) * 512],
                                start=(t == 0),
                                stop=(t == QH - 1),
                            )
                        esl = slice(eh * 512, (eh + 1) * 512)
                        if ec % 2 == 0:
                            nc.scalar.activation(ys[:, esl], yp[:], AF.Copy)
                        else:
                            nc.vector.tensor_copy(ys[:, esl], yp[:])
                    h0 = half * (D // 2)
                    nc.sync.dma_start(out_p[tsl, h0:h0 + D // 2], ys[:])

            if mc + 1 < MC:
                xt_cur = xt_nxt
                cs_cur = cs_nxt

    nc.finalize()
    return nc


def kernel(x, wq, wk, wv, wo, q_norm_w, k_norm_w, rope_cache, positions, cu_seqlens):
    global LAST_RESULT
    from concourse.bass_utils import run_bass_kernel_spmd  # noqa: PLC0415

    x = np.asarray(x, dtype=np.float32)
    wq = np.asarray(wq, dtype=np.float32)
    wk = np.asarray(wk, dtype=np.float32)
    wv = np.asarray(wv, dtype=np.float32)
    wo = np.asarray(wo, dtype=np.float32)
    q_norm_w = np.asarray(q_norm_w, dtype=np.float32)
    k_norm_w = np.asarray(k_norm_w, dtype=np.float32)
    rope_cache = np.asarray(rope_cache, dtype=np.float32)
    positions = np.asarray(positions)
    cu_seqlens = np.asarray(cu_seqlens)

    import ml_dtypes  # noqa: PLC0415

    bf = ml_dtypes.bfloat16

    # ---- host prep (shared) ----
    # x pretiled into (mc, g) groups of GD d-tiles: group (mc,g) = rows
    # [(mc*NG+g)*128, +128), cols di*512+c  <->  xT[(g*GD+di)*128+p, mc*512+c]
    xT = x[0].T.astype(bf)                       # [D, S]
    xt_host = np.ascontiguousarray(
        xT.reshape(NG, GD, 128, MC, 512).transpose(3, 0, 2, 1, 4)
        .reshape(MC * NG * 128, GD * 512)
    )

    pos = positions.reshape(-1)
    cs = rope_cache[pos]               # [S, HALF, 2]
    cosT = cs[:, :, 0].T               # [HALF, S]
    sinT = cs[:, :, 1].T
    cs1 = np.concatenate([cosT, cosT], axis=0)    # [128, S]
    cs2 = np.concatenate([-sinT, sinT], axis=0)

    def fold(w):
        w = w.reshape(HD, 1)
        wsw = np.concatenate([w[HALF:], w[:HALF]], axis=0)
        return cs1 * w, cs2 * wsw

    cs1q, cs2q = fold(q_norm_w)
    cs1k, cs2k = fold(k_norm_w)
    # per-chunk [128, 4*512]: rows mc*128+p, block ci at cols ci*512
    cs_host = np.ascontiguousarray(
        np.stack([cs1q, cs2q, cs1k, cs2k], axis=0).astype(bf)
        .reshape(4, 128, MC, 512).transpose(2, 1, 0, 3)
        .reshape(MC * 128, 4 * 512)
    )

    plan, mask_pack = _attention_plan(cu_seqlens)

    consts_bf = np.zeros((128, 2 * 128 + mask_pack.shape[1]), dtype=np.float32)
    consts_bf[:, 0:128] = 1.0
    consts_bf[:, 128:256] = np.eye(128, dtype=np.float32)
    consts_bf[:, 256:] = mask_pack
    consts_bf = consts_bf.astype(bf)

    # ---- per-core weight shards ----
    in_maps = []
    for c in range(NCORES):
        w_all = np.concatenate(
            [
                wq[c * QH * HD:(c + 1) * QH * HD],   # [512, D]
                wk[c * HD:(c + 1) * HD],             # [128, D]
                wv[c * HD:(c + 1) * HD],             # [128, D]
            ],
            axis=0,
        )  # [NO*128, D]
        # d-major pretile: (d, o) tile at rows d*128, cols o*128
        w_dmaj = (
            w_all.reshape(NO, 128, DT, 128).transpose(3, 2, 0, 1)
            .reshape(128, DT, NO * 128).transpose(1, 0, 2)
            .reshape(DT * 128, NO * 128).astype(bf)
        )
        # regroup into WG-d DMA groups: rows g*128+p, cols dj*(NO*128)+c
        w_host = np.ascontiguousarray(
            w_dmaj.reshape(DT // WG, WG, 128, NO * 128).transpose(0, 2, 1, 3)
            .reshape((DT // WG) * 128, WG * NO * 128)
        )
        wo_c = wo[:, c * QH * HD:(c + 1) * QH * HD].T  # [512, D]
        wo_host = np.ascontiguousarray(
            wo_c.reshape(QH, 128, D).transpose(1, 0, 2)
            .reshape(128, 4, QH * D // 4).transpose(1, 0, 2)
            .reshape(4 * 128, QH * D // 4).astype(bf)
        )
        in_maps.append(
            {
                "xT": xt_host,
                "w_qkv": w_host,
                "w_o": wo_host,
                "cs": cs_host,
                "consts_bf": consts_bf,
            }
        )

    nc = _build_graph(plan, mask_pack.shape[1])
    res = run_bass_kernel_spmd(nc, in_maps, list(range(NCORES)))
    LAST_RESULT = res

    out = res.results[0]["out"].astype(np.float32)
    for c in range(1, NCORES):
        out = out + res.results[c]["out"].astype(np.float32)
    return out.reshape(1, S, D)
